# revision 15
# baseline (speedup 1.0000x reference)
"""Grouped-Query Attention (B=1, L=4096, D=1024, 16 q-heads, 4 kv-heads, hd=64)
on 8 Trainium2 NeuronCores.

Sharding: core c owns q-heads {2c, 2c+1} and their shared kv-head c//2.
Each core computes Q/K/V projections for its heads from the full (replicated)
x, runs dense softmax attention for its 2 heads, and produces a partial
output projection  attn_heads @ Wo[head_rows]  of full shape [4096, 1024].
Host sums the 8 partials and adds bo (row-parallel all-reduce on host).

v2 structure (vs v1):
  - scores for the two heads run CONCURRENTLY in 64x128 PE row-tiles
    (T0: partitions 0-63, T8: 64-127) -- K^T duplicated on both halves,
    Q^T packed h0-top/h1-bottom.  2x effective score throughput.
  - Q projection computes both heads in ONE matmul chain (M=128).
  - PV uses the [q, d] output layout: stationary = P^T chunk [k128, q128],
    moving = V|1 [k128, 65].  Full 128 output partitions; the ones-column
    yields the softmax denominator per q ON the partition axis, so the
    epilogue is a cheap per-partition reciprocal + tensor_scalar multiply.
  - av PSUM bank holds 4 q-chunk accumulation regions; a C=1 zero matmul
    opens the bank (sets has_written everywhere) so all PV matmuls
    accumulate with start=False.
  - attn [q, d] is PE-transposed back to [d, q] for the out-projection.
  - software-pipelined emission: next block's Q-proj + first score group
    are emitted before the previous block's epilogue/out-proj; K/V
    projections are emitted just-in-time inside block 0; x^T DMA is
    split L-chunk-first so the pipeline starts after ~1/8 of the load.
"""

import os

os.environ.setdefault("MYCRO_LOCAL_CACHE", "1")

import numpy as np
import ml_dtypes

import concourse.bass as bass
import concourse.bacc as bacc
import concourse.mybir as mybir
from concourse.tile import TileContext
from concourse.bass_utils import run_bass_kernel_spmd

BF16 = mybir.dt.bfloat16
F32 = mybir.dt.float32
AF = mybir.ActivationFunctionType

D = 1024
L = 4096
NHEAD = 16
NKV = 4
HD = 64
NCORES = 8
HPC = NHEAD // NCORES  # 2 q heads per core
QB = 512               # q-block width
NQB = L // QB          # 8
KT = 128               # k-tile
NKT = L // KT          # 32
KG = 3                 # k-tiles per exp group
NG = (NKT + KG - 1) // KG  # 11 groups (10x3 + 1x2)
NF = D // 128          # 8 feature chunks
SCALE = 0.125          # 1/sqrt(64)

_CACHE = {}


def _build(has_bias):
    nc = bacc.Bacc("TRN2", target_bir_lowering=False, debug=False)

    xT = nc.declare_dram_parameter("xT", [D, L], BF16, isOutput=False)
    wq = nc.declare_dram_parameter("wq", [D, HPC * HD], BF16, isOutput=False)
    wk = nc.declare_dram_parameter("wk", [D, 2 * HD], BF16, isOutput=False)
    wv = nc.declare_dram_parameter("wv", [D, HD], BF16, isOutput=False)
    wo0 = nc.declare_dram_parameter("wo0", [HD, D], BF16, isOutput=False)
    wo1 = nc.declare_dram_parameter("wo1", [HD, D], BF16, isOutput=False)
    ident = nc.declare_dram_parameter("ident", [128, 128], BF16, isOutput=False)
    bq = nc.declare_dram_parameter("bq", [1, HPC * HD], BF16, isOutput=False)
    bk = nc.declare_dram_parameter("bk", [1, 2 * HD], BF16, isOutput=False)
    bv = nc.declare_dram_parameter("bv", [1, HD], BF16, isOutput=False)
    out = nc.declare_dram_parameter("out", [L, D], F32, isOutput=True)

    # group boundaries: (k0, gs)
    groups = []
    k = 0
    while k < NKT:
        gs = min(KG, NKT - k)
        groups.append((k, gs))
        k += gs

    with TileContext(nc) as tc:
        with (
            tc.tile_pool(name="sing", bufs=1) as sing,
            tc.tile_pool(name="ptp", bufs=2) as ptp,
            tc.tile_pool(name="atp", bufs=2) as atp,
            tc.tile_pool(name="rsp", bufs=2) as rsp,
            tc.tile_pool(name="obp", bufs=3) as obp,
            tc.tile_pool(name="psS", bufs=2, space="PSUM") as psS,
            tc.tile_pool(name="psV", bufs=1, space="PSUM") as psV,
        ):
            # ---- resident SBUF tensors ----
            xT_sb = sing.tile([128, NF, L], BF16)
            wq_sb = sing.tile([128, NF, HPC * HD], BF16)
            wk_sb = sing.tile([128, NF, 2 * HD], BF16)
            wv_sb = sing.tile([128, NF, HD], BF16)
            wo0_sb = sing.tile([HD, D], BF16)
            wo1_sb = sing.tile([HD, D], BF16)
            id_sb = sing.tile([128, 128], BF16)
            KT_sb = sing.tile([128, L], BF16)    # K^T duplicated on both halves
            QT_sb = sing.tile([128, L], BF16)    # h0 rows 0-63, h1 rows 64-127
            V_sb = sing.tile([128, NKT, HD + 1], BF16)  # col 64 = 1.0 (denom)
            zc_sb = sing.tile([1, 128], BF16)
            zr_sb = sing.tile([1, HPC * (HD + 1) * 2], BF16)  # >= 260 zeros
            if has_bias:
                bq_sb = sing.tile([1, HPC * HD], BF16)
                bk_sb = sing.tile([1, 2 * HD], BF16)
                bv_sb = sing.tile([1, HD], BF16)
                ones_b = sing.tile([1, QB], BF16)

            # x^T DMA, L-chunk-major so early k/q columns land first
            # (1024-wide chunks -> 2KB-per-partition descriptors)
            for lc in range(4):
                ls = slice(1024 * lc, 1024 * (lc + 1))
                for f in range(NF):
                    fs = slice(128 * f, 128 * (f + 1))
                    nc.sync.dma_start(out=xT_sb[:, f, ls], in_=xT[fs, ls])
                if lc == 0:
                    for f in range(NF):
                        fs = slice(128 * f, 128 * (f + 1))
                        nc.sync.dma_start(out=wq_sb[:, f, :], in_=wq[fs, :])
                        nc.sync.dma_start(out=wk_sb[:, f, :], in_=wk[fs, :])
                        nc.sync.dma_start(out=wv_sb[:, f, :], in_=wv[fs, :])
                    nc.sync.dma_start(out=wo0_sb, in_=wo0[:, :])
                    nc.sync.dma_start(out=wo1_sb, in_=wo1[:, :])
                    nc.sync.dma_start(out=id_sb, in_=ident[:, :])
                    if has_bias:
                        nc.sync.dma_start(out=bq_sb, in_=bq[:, :])
                        nc.sync.dma_start(out=bk_sb, in_=bk[:, :])
                        nc.sync.dma_start(out=bv_sb, in_=bv[:, :])
                        nc.gpsimd.memset(ones_b, 1.0)
            nc.gpsimd.memset(V_sb[:, :, HD], 1.0)
            nc.gpsimd.memset(zc_sb, 0.0)
            nc.gpsimd.memset(zr_sb, 0.0)

            # ---- projection emitters ----
            def emit_kproj(n):
                # K^T[128, 512] block n -- wk columns host-duplicated, so one
                # M=128 chain writes K^T to both partition halves directly
                ns = slice(QB * n, QB * (n + 1))
                kps = psS.tile([128, QB], F32, tag="st", name="kps")
                for f in range(NF):
                    nc.tensor.matmul(kps, wk_sb[:, f, :], xT_sb[:, f, ns],
                                     start=(f == 0),
                                     stop=(not has_bias and f == NF - 1))
                if has_bias:
                    nc.tensor.matmul(kps, bk_sb, ones_b, start=False, stop=True)
                nc.vector.tensor_copy(KT_sb[:, ns], kps)

            def emit_vproj(l):
                # V[128, 64] k-tile l (natural layout, k on partitions)
                ls = slice(KT * l, KT * (l + 1))
                vps = psS.tile([128, HD], F32, tag="st", name="vps")
                for f in range(NF):
                    nc.tensor.matmul(vps, xT_sb[:, f, ls], wv_sb[:, f, :],
                                     start=(f == 0),
                                     stop=(not has_bias and f == NF - 1))
                if has_bias:
                    nc.tensor.matmul(vps, ones_b[:, 0:KT], bv_sb,
                                     start=False, stop=True)
                nc.vector.tensor_copy(V_sb[:, l, 0:HD], vps)

            def emit_qproj(q):
                # Q^T[128, 512] both heads in one chain (unscaled; exp scales)
                qs = slice(QB * q, QB * (q + 1))
                qps = psS.tile([128, QB], F32, tag="st", name="qps")
                for f in range(NF):
                    nc.tensor.matmul(qps, wq_sb[:, f, :], xT_sb[:, f, qs],
                                     start=(f == 0),
                                     stop=(not has_bias and f == NF - 1))
                if has_bias:
                    nc.tensor.matmul(qps, bq_sb, ones_b, start=False, stop=True)
                nc.vector.tensor_copy(QT_sb[:, qs], qps)

            # live tiles
            st_tiles = {}  # (q, g, h) -> score tile
            cur_pt = {}   # (h) -> current exp'd tile
            cur_av = {}   # (h) -> av accumulation tile
            cur_atn = {}  # (h) -> normalized attn [q, d]
            cur_atT = {}  # (h) -> transposed attn [d, q]

            def emit_scores(q, g):
                qs = slice(QB * q, QB * (q + 1))
                k0, gs = groups[g]
                for h in range(HPC):
                    st = psS.tile([128, KG, QB], F32, tag="st", name=f"st{h}")
                    p = 64 * h
                    for j in range(gs):
                        ks = slice(KT * (k0 + j), KT * (k0 + j + 1))
                        nc.tensor.matmul(st[:, j, :], KT_sb[p:p + HD, ks],
                                         QT_sb[p:p + HD, qs],
                                         start=True, stop=True,
                                         tile_position=(p, 0))
                    st_tiles[(q, g, h)] = st

            def emit_exp(q, g):
                k0, gs = groups[g]
                for h in range(HPC):
                    pt = ptp.tile([128, KG, QB], BF16, tag=f"pt{h}",
                                  name=f"pt{h}")
                    st = st_tiles.pop((q, g, h))
                    nc.scalar.activation(pt[:, 0:gs, :], st[:, 0:gs, :],
                                         AF.Exp, scale=SCALE)
                    cur_pt[h] = pt

            def emit_zero_av():
                for h in range(HPC):
                    av = psV.tile([128, QB // KT, HD + 1], F32, tag=f"av{h}",
                                  name=f"av{h}")
                    nc.tensor.matmul(av[:, :, :], zc_sb,
                                     zr_sb[:, 0:(QB // KT) * (HD + 1)],
                                     start=True, stop=False,
                                     skip_group_check=True)
                    cur_av[h] = av

            def emit_pv(g):
                k0, gs = groups[g]
                for h in range(HPC):
                    av = cur_av[h]
                    pt = cur_pt[h]
                    for j in range(gs):
                        last = (k0 + j == NKT - 1)
                        for qc in range(QB // KT):
                            nc.tensor.matmul(
                                av[:, qc, :],
                                pt[:, j, KT * qc:KT * (qc + 1)],
                                V_sb[:, k0 + j, :],
                                start=False, stop=last,
                                skip_group_check=True)

            def emit_epilogue():
                # per-partition denom -> reciprocal -> scale -> transpose
                for h in range(HPC):
                    av = cur_av[h]
                    rsb = rsp.tile([128, QB // KT], F32, tag=f"rs{h}",
                                   name=f"rs{h}")
                    nc.vector.reciprocal(rsb, av[:, :, HD])
                    atn = atp.tile([128, QB // KT, HD], BF16, tag=f"at{h}",
                                   name=f"atn{h}")
                    for qc in range(QB // KT):
                        nc.vector.tensor_scalar_mul(
                            atn[:, qc, :], av[:, qc, 0:HD],
                            rsb[:, qc:qc + 1])
                    cur_atn[h] = atn
                for h in range(HPC):
                    psT = psV.tile([HD, QB // KT, KT], BF16, tag=f"av{h}",
                                   name=f"psT{h}")
                    for qc in range(QB // KT):
                        nc.tensor.transpose(psT[:, qc, :], cur_atn[h][:, qc, :],
                                            id_sb)
                    atT = atp.tile([HD, QB // KT, KT], BF16, tag=f"aT{h}",
                                   name=f"atT{h}")
                    nc.vector.tensor_copy(atT, psT)
                    cur_atT[h] = atT

            def emit_outproj(q, lcs):
                at0, at1 = cur_atT[0], cur_atT[1]
                for lc in lcs:
                    ops = psS.tile([128, 2, QB], F32, tag="st", name="ops")
                    osb = obp.tile([128, D], F32, tag="ob", name="osb")
                    for n in range(2):
                        ns = slice(QB * n, QB * (n + 1))
                        nc.tensor.matmul(ops[:, n, :], at0[:, lc, :],
                                         wo0_sb[:, ns], start=True, stop=False)
                        nc.tensor.matmul(ops[:, n, :], at1[:, lc, :],
                                         wo1_sb[:, ns], start=False, stop=True)
                    nc.vector.tensor_copy(osb, ops)
                    nc.sync.dma_start(
                        out=out[QB * q + KT * lc:QB * q + KT * (lc + 1), :],
                        in_=osb)

            # ---- software-pipelined emission ----
            # prologue: enough K/V for group 0, Q-proj 0, scores(0, 0)
            emit_kproj(0)                      # k-tiles 0-3
            for l in range(3):
                emit_vproj(l)
            kdone, vdone = 1, 3
            emit_qproj(0)
            emit_scores(0, 0)

            # holds (epi_q, atT deferral) state
            for q in range(NQB):
                for g in range(NG):
                    emit_exp(q, g)
                    # JIT K/V projections during block 0
                    if q == 0 and g + 1 < NG:
                        k1, gs1 = groups[g + 1]
                        need_k = min((k1 + gs1 + 3) // 4, NQB)
                        while kdone < need_k:
                            emit_kproj(kdone)
                            kdone += 1
                        while vdone < min(k1 + gs1, NKT):
                            emit_vproj(vdone)
                            vdone += 1
                    if g + 1 < NG:
                        emit_scores(q, g + 1)
                    if q + 1 < NQB:
                        if g == NG - 3:
                            emit_qproj(q + 1)
                        elif g == NG - 2:
                            emit_scores(q + 1, 0)
                    if g == 0:
                        if q > 0:
                            emit_epilogue()
                        emit_zero_av()
                    emit_pv(g)
                    if q > 0 and 1 <= g <= 4:
                        emit_outproj(q - 1, (g - 1,))
            emit_epilogue()
            emit_outproj(NQB - 1, (0, 1, 2, 3))
    nc.finalize()
    return nc


def _prep_inputs(x, Wq, bq, Wk, bk, Wv, bv, Wo, bo):
    bf = ml_dtypes.bfloat16
    xT = np.ascontiguousarray(np.asarray(x, dtype=np.float32)[0].T).astype(bf)
    Wq = np.asarray(Wq, dtype=np.float32)
    Wk = np.asarray(Wk, dtype=np.float32)
    Wv = np.asarray(Wv, dtype=np.float32)
    Wo = np.asarray(Wo, dtype=np.float32)
    bq = np.asarray(bq, dtype=np.float32)
    bk = np.asarray(bk, dtype=np.float32)
    bv = np.asarray(bv, dtype=np.float32)
    has_bias = bool(np.any(bq) or np.any(bk) or np.any(bv))
    ident = np.eye(128, dtype=np.float32).astype(bf)
    in_maps = []
    for c in range(NCORES):
        qsl = slice(HPC * HD * c, HPC * HD * (c + 1))   # this core's q-head cols
        kv = c // 2                                     # its kv head
        ksl = slice(HD * kv, HD * (kv + 1))
        wk_c = np.ascontiguousarray(Wk[:, ksl])
        in_maps.append({
            "xT": xT,
            "wq": np.ascontiguousarray(Wq[:, qsl]).astype(bf),
            "wk": np.concatenate([wk_c, wk_c], axis=1).astype(bf),
            "wv": np.ascontiguousarray(Wv[:, ksl]).astype(bf),
            "wo0": np.ascontiguousarray(
                Wo[HPC * HD * c:HPC * HD * c + HD, :]).astype(bf),
            "wo1": np.ascontiguousarray(
                Wo[HPC * HD * c + HD:HPC * HD * (c + 1), :]).astype(bf),
            "ident": ident,
            "bq": bq[qsl].reshape(1, -1).astype(bf),
            "bk": np.concatenate([bk[ksl], bk[ksl]]).reshape(1, -1).astype(bf),
            "bv": bv[ksl].reshape(1, -1).astype(bf),
        })
    return in_maps, has_bias


def run(inputs, trace=False):
    in_maps, has_bias = _prep_inputs(**inputs)
    key = ("nc", has_bias)
    if key not in _CACHE:
        _CACHE[key] = _build(has_bias)
    nc = _CACHE[key]
    res = run_bass_kernel_spmd(nc, in_maps, list(range(NCORES)), trace=trace)
    bo = np.asarray(inputs["bo"], dtype=np.float32)
    acc = np.zeros((L, D), dtype=np.float32)
    for r in res.results:
        acc += np.asarray(r["out"], dtype=np.float32)
    out = (acc + bo).reshape(1, L, D)
    return out, res


def kernel(**inputs):
    out, _ = run(inputs, trace=False)
    return out


# revision 17
# speedup vs baseline: 1.0611x; 1.0611x over previous
"""Grouped-Query Attention (B=1, L=4096, D=1024, 16 q-heads, 4 kv-heads, hd=64)
on 8 Trainium2 NeuronCores.

Sharding: core c owns q-heads {2c, 2c+1} and their shared kv-head c//2.
Each core computes Q/K/V projections for its heads from the full (replicated)
x, runs dense softmax attention for its 2 heads, and produces a partial
output projection  attn_heads @ Wo[head_rows]  of full shape [4096, 1024].
Host sums the 8 partials and adds bo (row-parallel all-reduce on host).

v2 structure (vs v1):
  - scores for the two heads run CONCURRENTLY in 64x128 PE row-tiles
    (T0: partitions 0-63, T8: 64-127) -- K^T duplicated on both halves,
    Q^T packed h0-top/h1-bottom.  2x effective score throughput.
  - Q projection computes both heads in ONE matmul chain (M=128).
  - PV uses the [q, d] output layout: stationary = P^T chunk [k128, q128],
    moving = V|1 [k128, 65].  Full 128 output partitions; the ones-column
    yields the softmax denominator per q ON the partition axis, so the
    epilogue is a cheap per-partition reciprocal + tensor_scalar multiply.
  - av PSUM bank holds 4 q-chunk accumulation regions; a C=1 zero matmul
    opens the bank (sets has_written everywhere) so all PV matmuls
    accumulate with start=False.
  - attn [q, d] is PE-transposed back to [d, q] for the out-projection.
  - software-pipelined emission: next block's Q-proj + first score group
    are emitted before the previous block's epilogue/out-proj; K/V
    projections are emitted just-in-time inside block 0; x^T DMA is
    split L-chunk-first so the pipeline starts after ~1/8 of the load.
"""

import os

os.environ.setdefault("MYCRO_LOCAL_CACHE", "1")

import numpy as np
import ml_dtypes

import concourse.bass as bass
import concourse.bacc as bacc
import concourse.mybir as mybir
from concourse.tile import TileContext
from concourse.bass_utils import run_bass_kernel_spmd

BF16 = mybir.dt.bfloat16
F32 = mybir.dt.float32
AF = mybir.ActivationFunctionType

D = 1024
L = 4096
NHEAD = 16
NKV = 4
HD = 64
NCORES = 8
HPC = NHEAD // NCORES  # 2 q heads per core
QB = 512               # q-block width
NQB = L // QB          # 8
KT = 128               # k-tile
NKT = L // KT          # 32
KG = 3                 # k-tiles per exp group
NG = (NKT + KG - 1) // KG  # 11 groups (10x3 + 1x2)
NF = D // 128          # 8 feature chunks
SCALE = 0.125          # 1/sqrt(64)

_CACHE = {}


def _build(has_bias):
    nc = bacc.Bacc("TRN2", target_bir_lowering=False, debug=False)

    xT = nc.declare_dram_parameter("xT", [D, L], BF16, isOutput=False)
    wq = nc.declare_dram_parameter("wq", [D, HPC * HD], BF16, isOutput=False)
    wk = nc.declare_dram_parameter("wk", [D, 2 * HD], BF16, isOutput=False)
    wv = nc.declare_dram_parameter("wv", [D, HD], BF16, isOutput=False)
    wo0 = nc.declare_dram_parameter("wo0", [HD, D], BF16, isOutput=False)
    wo1 = nc.declare_dram_parameter("wo1", [HD, D], BF16, isOutput=False)
    ident = nc.declare_dram_parameter("ident", [128, 128], BF16, isOutput=False)
    bq = nc.declare_dram_parameter("bq", [1, HPC * HD], BF16, isOutput=False)
    bk = nc.declare_dram_parameter("bk", [1, 2 * HD], BF16, isOutput=False)
    bv = nc.declare_dram_parameter("bv", [1, HD], BF16, isOutput=False)
    out = nc.declare_dram_parameter("out", [L, D], F32, isOutput=True)

    # group boundaries: (k0, gs)
    groups = []
    k = 0
    while k < NKT:
        gs = min(KG, NKT - k)
        groups.append((k, gs))
        k += gs

    with TileContext(nc) as tc:
        with (
            tc.tile_pool(name="sing", bufs=1) as sing,
            tc.tile_pool(name="ptp", bufs=2) as ptp,
            tc.tile_pool(name="atp", bufs=2) as atp,
            tc.tile_pool(name="rsp", bufs=2) as rsp,
            tc.tile_pool(name="obp", bufs=3) as obp,
            tc.tile_pool(name="psS", bufs=2, space="PSUM") as psS,
            tc.tile_pool(name="psV", bufs=1, space="PSUM") as psV,
        ):
            # ---- resident SBUF tensors ----
            xT_sb = sing.tile([128, NF, L], BF16)
            wq_sb = sing.tile([128, NF, HPC * HD], BF16)
            wk_sb = sing.tile([128, NF, 2 * HD], BF16)
            wv_sb = sing.tile([128, NF, HD], BF16)
            wo0_sb = sing.tile([HD, D], BF16)
            wo1_sb = sing.tile([HD, D], BF16)
            id_sb = sing.tile([128, 128], BF16)
            KT_sb = sing.tile([128, L], BF16)    # K^T duplicated on both halves
            QT_sb = sing.tile([128, L], BF16)    # h0 rows 0-63, h1 rows 64-127
            V_sb = sing.tile([128, NKT, HD + 1], BF16)  # col 64 = 1.0 (denom)
            zc_sb = sing.tile([1, 128], BF16)
            zr_sb = sing.tile([1, HPC * (HD + 1) * 2], BF16)  # >= 260 zeros
            if has_bias:
                bq_sb = sing.tile([1, HPC * HD], BF16)
                bk_sb = sing.tile([1, 2 * HD], BF16)
                bv_sb = sing.tile([1, HD], BF16)
                ones_b = sing.tile([1, QB], BF16)

            # x^T DMA, L-chunk-major so early k/q columns land first.
            # Each transfer covers 64 partitions so two DMA queues work per
            # f-chunk; first chunk is 512 wide to minimize time-to-first-matmul
            def dma_xt(ls):
                for f in range(NF):
                    for pp in range(2):
                        fs = slice(128 * f + 64 * pp, 128 * f + 64 * (pp + 1))
                        nc.sync.dma_start(out=xT_sb[64 * pp:64 * (pp + 1), f, ls],
                                          in_=xT[fs, ls])

            for lc in range(5):
                ls = slice(0, 512) if lc == 0 else \
                    slice(512 + 896 * (lc - 1), 512 + 896 * lc)
                dma_xt(ls)
                if lc == 0:
                    for f in range(NF):
                        fs = slice(128 * f, 128 * (f + 1))
                        nc.sync.dma_start(out=wq_sb[:, f, :], in_=wq[fs, :])
                        nc.sync.dma_start(out=wk_sb[:, f, :], in_=wk[fs, :])
                        nc.sync.dma_start(out=wv_sb[:, f, :], in_=wv[fs, :])
                    nc.sync.dma_start(out=wo0_sb, in_=wo0[:, :])
                    nc.sync.dma_start(out=wo1_sb, in_=wo1[:, :])
                    nc.sync.dma_start(out=id_sb, in_=ident[:, :])
                    if has_bias:
                        nc.sync.dma_start(out=bq_sb, in_=bq[:, :])
                        nc.sync.dma_start(out=bk_sb, in_=bk[:, :])
                        nc.sync.dma_start(out=bv_sb, in_=bv[:, :])
                        nc.gpsimd.memset(ones_b, 1.0)
            nc.gpsimd.memset(V_sb[:, :, HD], 1.0)
            nc.gpsimd.memset(zc_sb, 0.0)
            nc.gpsimd.memset(zr_sb, 0.0)

            # ---- projection emitters ----
            def emit_kproj(n):
                # K^T[128, 512] block n -- wk columns host-duplicated, so one
                # M=128 chain writes K^T to both partition halves directly
                ns = slice(QB * n, QB * (n + 1))
                kps = psS.tile([128, QB], F32, tag="st", name="kps")
                for f in range(NF):
                    nc.tensor.matmul(kps, wk_sb[:, f, :], xT_sb[:, f, ns],
                                     start=(f == 0),
                                     stop=(not has_bias and f == NF - 1))
                if has_bias:
                    nc.tensor.matmul(kps, bk_sb, ones_b, start=False, stop=True)
                nc.vector.tensor_copy(KT_sb[:, ns], kps)

            def emit_vproj(l):
                # V[128, 64] k-tile l (natural layout, k on partitions)
                ls = slice(KT * l, KT * (l + 1))
                vps = psS.tile([128, HD], F32, tag="st", name="vps")
                for f in range(NF):
                    nc.tensor.matmul(vps, xT_sb[:, f, ls], wv_sb[:, f, :],
                                     start=(f == 0),
                                     stop=(not has_bias and f == NF - 1))
                if has_bias:
                    nc.tensor.matmul(vps, ones_b[:, 0:KT], bv_sb,
                                     start=False, stop=True)
                nc.vector.tensor_copy(V_sb[:, l, 0:HD], vps)

            def emit_qproj(q):
                # Q^T[128, 512] both heads in one chain (unscaled; exp scales)
                qs = slice(QB * q, QB * (q + 1))
                qps = psS.tile([128, QB], F32, tag="st", name="qps")
                for f in range(NF):
                    nc.tensor.matmul(qps, wq_sb[:, f, :], xT_sb[:, f, qs],
                                     start=(f == 0),
                                     stop=(not has_bias and f == NF - 1))
                if has_bias:
                    nc.tensor.matmul(qps, bq_sb, ones_b, start=False, stop=True)
                nc.vector.tensor_copy(QT_sb[:, qs], qps)

            # live tiles
            st_tiles = {}  # (q, g, h) -> score tile
            cur_pt = {}   # (h) -> current exp'd tile
            cur_av = {}   # (h) -> av accumulation tile
            cur_atn = {}  # (h) -> normalized attn [q, d]
            cur_atT = {}  # (h) -> transposed attn [d, q]

            def emit_scores(q, g):
                qs = slice(QB * q, QB * (q + 1))
                k0, gs = groups[g]
                for h in range(HPC):
                    st = psS.tile([128, KG, QB], F32, tag="st", name=f"st{h}")
                    p = 64 * h
                    for j in range(gs):
                        ks = slice(KT * (k0 + j), KT * (k0 + j + 1))
                        nc.tensor.matmul(st[:, j, :], KT_sb[p:p + HD, ks],
                                         QT_sb[p:p + HD, qs],
                                         start=True, stop=True,
                                         tile_position=(p, 0))
                    st_tiles[(q, g, h)] = st

            def emit_exp(q, g):
                k0, gs = groups[g]
                for h in range(HPC):
                    pt = ptp.tile([128, KG, QB], BF16, tag=f"pt{h}",
                                  name=f"pt{h}")
                    st = st_tiles.pop((q, g, h))
                    nc.scalar.activation(pt[:, 0:gs, :], st[:, 0:gs, :],
                                         AF.Exp, scale=SCALE)
                    cur_pt[h] = pt

            def emit_zero_av():
                for h in range(HPC):
                    av = psV.tile([128, QB // KT, HD + 1], F32, tag=f"av{h}",
                                  name=f"av{h}")
                    nc.tensor.matmul(av[:, :, :], zc_sb,
                                     zr_sb[:, 0:(QB // KT) * (HD + 1)],
                                     start=True, stop=False,
                                     skip_group_check=True)
                    cur_av[h] = av

            def emit_pv(g):
                k0, gs = groups[g]
                for h in range(HPC):
                    av = cur_av[h]
                    pt = cur_pt[h]
                    for j in range(gs):
                        last = (k0 + j == NKT - 1)
                        for qc in range(QB // KT):
                            nc.tensor.matmul(
                                av[:, qc, :],
                                pt[:, j, KT * qc:KT * (qc + 1)],
                                V_sb[:, k0 + j, :],
                                start=False, stop=last,
                                skip_group_check=True)

            def emit_epilogue():
                # per-partition denom -> reciprocal -> scale -> transpose
                for h in range(HPC):
                    av = cur_av[h]
                    rsb = rsp.tile([128, QB // KT], F32, tag=f"rs{h}",
                                   name=f"rs{h}")
                    nc.vector.reciprocal(rsb, av[:, :, HD])
                    atn = atp.tile([128, QB // KT, HD], BF16, tag=f"at{h}",
                                   name=f"atn{h}")
                    for qc in range(QB // KT):
                        nc.vector.tensor_scalar_mul(
                            atn[:, qc, :], av[:, qc, 0:HD],
                            rsb[:, qc:qc + 1])
                    cur_atn[h] = atn
                for h in range(HPC):
                    psT = psV.tile([HD, QB // KT, KT], BF16, tag=f"av{h}",
                                   name=f"psT{h}")
                    for qc in range(QB // KT):
                        nc.tensor.transpose(psT[:, qc, :], cur_atn[h][:, qc, :],
                                            id_sb)
                    atT = atp.tile([HD, QB // KT, KT], BF16, tag=f"aT{h}",
                                   name=f"atT{h}")
                    nc.vector.tensor_copy(atT, psT)
                    cur_atT[h] = atT

            def emit_outproj(q, lcs):
                at0, at1 = cur_atT[0], cur_atT[1]
                for lc in lcs:
                    ops = psS.tile([128, 2, QB], F32, tag="st", name="ops")
                    osb = obp.tile([128, D], F32, tag="ob", name="osb")
                    for n in range(2):
                        ns = slice(QB * n, QB * (n + 1))
                        nc.tensor.matmul(ops[:, n, :], at0[:, lc, :],
                                         wo0_sb[:, ns], start=True, stop=False)
                        nc.tensor.matmul(ops[:, n, :], at1[:, lc, :],
                                         wo1_sb[:, ns], start=False, stop=True)
                    nc.vector.tensor_copy(osb, ops)
                    nc.sync.dma_start(
                        out=out[QB * q + KT * lc:QB * q + KT * (lc + 1), :],
                        in_=osb)

            # ---- software-pipelined emission ----
            # prologue: enough K/V for group 0, Q-proj 0, scores(0, 0)
            emit_kproj(0)                      # k-tiles 0-3
            for l in range(3):
                emit_vproj(l)
            kdone, vdone = 1, 3
            emit_qproj(0)
            emit_scores(0, 0)

            # holds (epi_q, atT deferral) state
            for q in range(NQB):
                for g in range(NG):
                    emit_exp(q, g)
                    # JIT K/V projections during block 0
                    if q == 0 and g + 1 < NG:
                        k1, gs1 = groups[g + 1]
                        need_k = min((k1 + gs1 + 3) // 4, NQB)
                        while kdone < need_k:
                            emit_kproj(kdone)
                            kdone += 1
                        while vdone < min(k1 + gs1, NKT):
                            emit_vproj(vdone)
                            vdone += 1
                    if g + 1 < NG:
                        emit_scores(q, g + 1)
                    if g == 0:
                        if q > 0:
                            emit_epilogue()
                        emit_zero_av()
                    emit_pv(g)
                    if q > 0 and 1 <= g <= 4:
                        emit_outproj(q - 1, (g - 1,))
                    if q + 1 < NQB:
                        if g == 6:
                            emit_qproj(q + 1)
                        elif g == NG - 1:
                            emit_scores(q + 1, 0)
            emit_epilogue()
            emit_outproj(NQB - 1, (0, 1, 2, 3))
    nc.finalize()
    return nc


def _prep_inputs(x, Wq, bq, Wk, bk, Wv, bv, Wo, bo):
    bf = ml_dtypes.bfloat16
    xT = np.ascontiguousarray(np.asarray(x, dtype=np.float32)[0].T).astype(bf)
    Wq = np.asarray(Wq, dtype=np.float32)
    Wk = np.asarray(Wk, dtype=np.float32)
    Wv = np.asarray(Wv, dtype=np.float32)
    Wo = np.asarray(Wo, dtype=np.float32)
    bq = np.asarray(bq, dtype=np.float32)
    bk = np.asarray(bk, dtype=np.float32)
    bv = np.asarray(bv, dtype=np.float32)
    has_bias = bool(np.any(bq) or np.any(bk) or np.any(bv))
    ident = np.eye(128, dtype=np.float32).astype(bf)
    in_maps = []
    for c in range(NCORES):
        qsl = slice(HPC * HD * c, HPC * HD * (c + 1))   # this core's q-head cols
        kv = c // 2                                     # its kv head
        ksl = slice(HD * kv, HD * (kv + 1))
        wk_c = np.ascontiguousarray(Wk[:, ksl])
        in_maps.append({
            "xT": xT,
            "wq": np.ascontiguousarray(Wq[:, qsl]).astype(bf),
            "wk": np.concatenate([wk_c, wk_c], axis=1).astype(bf),
            "wv": np.ascontiguousarray(Wv[:, ksl]).astype(bf),
            "wo0": np.ascontiguousarray(
                Wo[HPC * HD * c:HPC * HD * c + HD, :]).astype(bf),
            "wo1": np.ascontiguousarray(
                Wo[HPC * HD * c + HD:HPC * HD * (c + 1), :]).astype(bf),
            "ident": ident,
            "bq": bq[qsl].reshape(1, -1).astype(bf),
            "bk": np.concatenate([bk[ksl], bk[ksl]]).reshape(1, -1).astype(bf),
            "bv": bv[ksl].reshape(1, -1).astype(bf),
        })
    return in_maps, has_bias


def run(inputs, trace=False):
    in_maps, has_bias = _prep_inputs(**inputs)
    key = ("nc", has_bias)
    if key not in _CACHE:
        _CACHE[key] = _build(has_bias)
    nc = _CACHE[key]
    res = run_bass_kernel_spmd(nc, in_maps, list(range(NCORES)), trace=trace)
    bo = np.asarray(inputs["bo"], dtype=np.float32)
    acc = np.zeros((L, D), dtype=np.float32)
    for r in res.results:
        acc += np.asarray(r["out"], dtype=np.float32)
    out = (acc + bo).reshape(1, L, D)
    return out, res


def kernel(**inputs):
    out, _ = run(inputs, trace=False)
    return out


# revision 19
# speedup vs baseline: 1.1273x; 1.0624x over previous
"""Grouped-Query Attention (B=1, L=4096, D=1024, 16 q-heads, 4 kv-heads, hd=64)
on 8 Trainium2 NeuronCores.

Sharding: core c owns q-heads {2c, 2c+1} and their shared kv-head c//2.
Each core computes Q/K/V projections for its heads from the full (replicated)
x, runs dense softmax attention for its 2 heads, and produces a partial
output projection  attn_heads @ Wo[head_rows]  of full shape [4096, 1024].
Host sums the 8 partials and adds bo (row-parallel all-reduce on host).

v2 structure (vs v1):
  - scores for the two heads run CONCURRENTLY in 64x128 PE row-tiles
    (T0: partitions 0-63, T8: 64-127) -- K^T duplicated on both halves,
    Q^T packed h0-top/h1-bottom.  2x effective score throughput.
  - Q projection computes both heads in ONE matmul chain (M=128).
  - PV uses the [q, d] output layout: stationary = P^T chunk [k128, q128],
    moving = V|1 [k128, 65].  Full 128 output partitions; the ones-column
    yields the softmax denominator per q ON the partition axis, so the
    epilogue is a cheap per-partition reciprocal + tensor_scalar multiply.
  - av PSUM bank holds 4 q-chunk accumulation regions; a C=1 zero matmul
    opens the bank (sets has_written everywhere) so all PV matmuls
    accumulate with start=False.
  - attn [q, d] is PE-transposed back to [d, q] for the out-projection.
  - software-pipelined emission: next block's Q-proj + first score group
    are emitted before the previous block's epilogue/out-proj; K/V
    projections are emitted just-in-time inside block 0; x^T DMA is
    split L-chunk-first so the pipeline starts after ~1/8 of the load.
"""

import os

os.environ.setdefault("MYCRO_LOCAL_CACHE", "1")

import numpy as np
import ml_dtypes

import concourse.bass as bass
import concourse.bacc as bacc
import concourse.mybir as mybir
from concourse.tile import TileContext
from concourse.bass_utils import run_bass_kernel_spmd

BF16 = mybir.dt.bfloat16
F32 = mybir.dt.float32
AF = mybir.ActivationFunctionType

D = 1024
L = 4096
NHEAD = 16
NKV = 4
HD = 64
NCORES = 8
HPC = NHEAD // NCORES  # 2 q heads per core
QB = 512               # q-block width
NQB = L // QB          # 8
KT = 128               # k-tile
NKT = L // KT          # 32
KG = 3                 # k-tiles per exp group
NG = (NKT + KG - 1) // KG  # 11 groups (10x3 + 1x2)
NF = D // 128          # 8 feature chunks
SCALE = 0.125          # 1/sqrt(64)

_CACHE = {}


def _build(has_bias):
    nc = bacc.Bacc("TRN2", target_bir_lowering=False, debug=False)

    xT = nc.declare_dram_parameter("xT", [D, L], BF16, isOutput=False)
    wq = nc.declare_dram_parameter("wq", [D, HPC * HD], BF16, isOutput=False)
    wk = nc.declare_dram_parameter("wk", [D, 2 * HD], BF16, isOutput=False)
    wv = nc.declare_dram_parameter("wv", [D, HD], BF16, isOutput=False)
    wo0 = nc.declare_dram_parameter("wo0", [HD, D], BF16, isOutput=False)
    wo1 = nc.declare_dram_parameter("wo1", [HD, D], BF16, isOutput=False)
    ident = nc.declare_dram_parameter("ident", [128, 128], BF16, isOutput=False)
    bq = nc.declare_dram_parameter("bq", [1, HPC * HD], BF16, isOutput=False)
    bk = nc.declare_dram_parameter("bk", [1, 2 * HD], BF16, isOutput=False)
    bv = nc.declare_dram_parameter("bv", [1, HD], BF16, isOutput=False)
    out = nc.declare_dram_parameter("out", [L, D], F32, isOutput=True)

    # group boundaries: (k0, gs)
    groups = []
    k = 0
    while k < NKT:
        gs = min(KG, NKT - k)
        groups.append((k, gs))
        k += gs

    with TileContext(nc) as tc:
        with (
            tc.tile_pool(name="sing", bufs=1) as sing,
            tc.tile_pool(name="ptp", bufs=2) as ptp,
            tc.tile_pool(name="atp", bufs=2) as atp,
            tc.tile_pool(name="rsp", bufs=2) as rsp,
            tc.tile_pool(name="obp", bufs=3) as obp,
            tc.tile_pool(name="psS", bufs=2, space="PSUM") as psS,
            tc.tile_pool(name="psV", bufs=1, space="PSUM") as psV,
        ):
            # ---- resident SBUF tensors ----
            xT_sb = sing.tile([128, NF, L], BF16)
            wq_sb = sing.tile([128, NF, HPC * HD], BF16)
            wk_sb = sing.tile([128, NF, 2 * HD], BF16)
            wv_sb = sing.tile([128, NF, HD], BF16)
            wo0_sb = sing.tile([HD, D], BF16)
            wo1_sb = sing.tile([HD, D], BF16)
            id_sb = sing.tile([128, 128], BF16)
            KT_sb = sing.tile([128, L], BF16)    # K^T duplicated on both halves
            QT_sb = sing.tile([128, L], BF16)    # h0 rows 0-63, h1 rows 64-127
            V_sb = sing.tile([128, NKT, HD + 1], BF16)  # col 64 = 1.0 (denom)
            zc_sb = sing.tile([1, 128], BF16)
            zr_sb = sing.tile([1, HPC * (HD + 1) * 2], BF16)  # >= 260 zeros
            if has_bias:
                bq_sb = sing.tile([1, HPC * HD], BF16)
                bk_sb = sing.tile([1, 2 * HD], BF16)
                bv_sb = sing.tile([1, HD], BF16)
                ones_b = sing.tile([1, QB], BF16)

            # x^T DMA, L-chunk-major so early k/q columns land first
            for lc in range(NQB):
                ls = slice(QB * lc, QB * (lc + 1))
                for f in range(NF):
                    fs = slice(128 * f, 128 * (f + 1))
                    nc.sync.dma_start(out=xT_sb[:, f, ls], in_=xT[fs, ls])
                if lc == 0:
                    for f in range(NF):
                        fs = slice(128 * f, 128 * (f + 1))
                        nc.sync.dma_start(out=wq_sb[:, f, :], in_=wq[fs, :])
                        nc.sync.dma_start(out=wk_sb[:, f, :], in_=wk[fs, :])
                        nc.sync.dma_start(out=wv_sb[:, f, :], in_=wv[fs, :])
                    nc.sync.dma_start(out=wo0_sb, in_=wo0[:, :])
                    nc.sync.dma_start(out=wo1_sb, in_=wo1[:, :])
                    nc.sync.dma_start(out=id_sb, in_=ident[:, :])
                    if has_bias:
                        nc.sync.dma_start(out=bq_sb, in_=bq[:, :])
                        nc.sync.dma_start(out=bk_sb, in_=bk[:, :])
                        nc.sync.dma_start(out=bv_sb, in_=bv[:, :])
                        nc.gpsimd.memset(ones_b, 1.0)
            nc.gpsimd.memset(V_sb[:, :, HD], 1.0)
            nc.gpsimd.memset(zc_sb, 0.0)
            nc.gpsimd.memset(zr_sb, 0.0)

            # ---- projection emitters ----
            def emit_kproj(n):
                # K^T[128, 512] block n -- wk columns host-duplicated, so one
                # M=128 chain writes K^T to both partition halves directly
                ns = slice(QB * n, QB * (n + 1))
                kps = psS.tile([128, QB], F32, tag="st", name="kps")
                for f in range(NF):
                    nc.tensor.matmul(kps, wk_sb[:, f, :], xT_sb[:, f, ns],
                                     start=(f == 0),
                                     stop=(not has_bias and f == NF - 1))
                if has_bias:
                    nc.tensor.matmul(kps, bk_sb, ones_b, start=False, stop=True)
                nc.vector.tensor_copy(KT_sb[:, ns], kps)

            def emit_vproj(l):
                # V[128, 64] k-tile l (natural layout, k on partitions)
                ls = slice(KT * l, KT * (l + 1))
                vps = psS.tile([128, HD], F32, tag="st", name="vps")
                for f in range(NF):
                    nc.tensor.matmul(vps, xT_sb[:, f, ls], wv_sb[:, f, :],
                                     start=(f == 0),
                                     stop=(not has_bias and f == NF - 1))
                if has_bias:
                    nc.tensor.matmul(vps, ones_b[:, 0:KT], bv_sb,
                                     start=False, stop=True)
                nc.vector.tensor_copy(V_sb[:, l, 0:HD], vps)

            def emit_qproj(q):
                # Q^T[128, 512] both heads in one chain (unscaled; exp scales)
                qs = slice(QB * q, QB * (q + 1))
                qps = psS.tile([128, QB], F32, tag="st", name="qps")
                for f in range(NF):
                    nc.tensor.matmul(qps, wq_sb[:, f, :], xT_sb[:, f, qs],
                                     start=(f == 0),
                                     stop=(not has_bias and f == NF - 1))
                if has_bias:
                    nc.tensor.matmul(qps, bq_sb, ones_b, start=False, stop=True)
                nc.vector.tensor_copy(QT_sb[:, qs], qps)

            # live tiles
            st_tiles = {}  # (q, g, h) -> score tile
            cur_pt = {}   # (h) -> current exp'd tile
            cur_av = {}   # (h) -> av accumulation tile
            cur_atn = {}  # (h) -> normalized attn [q, d]
            cur_atT = {}  # (h) -> transposed attn [d, q]

            def emit_scores(q, g):
                qs = slice(QB * q, QB * (q + 1))
                k0, gs = groups[g]
                for h in range(HPC):
                    st = psS.tile([128, KG, QB], F32, tag="st", name=f"st{h}")
                    p = 64 * h
                    for j in range(gs):
                        ks = slice(KT * (k0 + j), KT * (k0 + j + 1))
                        nc.tensor.matmul(st[:, j, :], KT_sb[p:p + HD, ks],
                                         QT_sb[p:p + HD, qs],
                                         start=True, stop=True,
                                         tile_position=(p, 0))
                    st_tiles[(q, g, h)] = st

            def emit_exp(q, g):
                k0, gs = groups[g]
                for h in range(HPC):
                    pt = ptp.tile([128, KG, QB], BF16, tag=f"pt{h}",
                                  name=f"pt{h}")
                    st = st_tiles.pop((q, g, h))
                    nc.scalar.activation(pt[:, 0:gs, :], st[:, 0:gs, :],
                                         AF.Exp, scale=SCALE)
                    cur_pt[h] = pt

            def emit_zero_av():
                for h in range(HPC):
                    av = psV.tile([128, QB // KT, HD + 1], F32, tag=f"av{h}",
                                  name=f"av{h}")
                    nc.tensor.matmul(av[:, :, :], zc_sb,
                                     zr_sb[:, 0:(QB // KT) * (HD + 1)],
                                     start=True, stop=False,
                                     skip_group_check=True)
                    cur_av[h] = av

            def emit_pv(g):
                k0, gs = groups[g]
                for h in range(HPC):
                    av = cur_av[h]
                    pt = cur_pt[h]
                    for j in range(gs):
                        last = (k0 + j == NKT - 1)
                        for qc in range(QB // KT):
                            nc.tensor.matmul(
                                av[:, qc, :],
                                pt[:, j, KT * qc:KT * (qc + 1)],
                                V_sb[:, k0 + j, :],
                                start=False, stop=last,
                                skip_group_check=True)

            def emit_epilogue():
                # per-partition denom -> reciprocal -> scale -> transpose
                for h in range(HPC):
                    av = cur_av[h]
                    rsb = rsp.tile([128, QB // KT], F32, tag=f"rs{h}",
                                   name=f"rs{h}")
                    nc.vector.reciprocal(rsb, av[:, :, HD])
                    atn = atp.tile([128, QB // KT, HD], BF16, tag=f"at{h}",
                                   name=f"atn{h}")
                    for qc in range(QB // KT):
                        nc.vector.tensor_scalar_mul(
                            atn[:, qc, :], av[:, qc, 0:HD],
                            rsb[:, qc:qc + 1])
                    cur_atn[h] = atn
                for h in range(HPC):
                    psT = psV.tile([HD, QB // KT, KT], BF16, tag=f"av{h}",
                                   name=f"psT{h}")
                    for qc in range(QB // KT):
                        nc.tensor.transpose(psT[:, qc, :], cur_atn[h][:, qc, :],
                                            id_sb)
                    atT = atp.tile([HD, QB // KT, KT], BF16, tag=f"aT{h}",
                                   name=f"atT{h}")
                    nc.vector.tensor_copy(atT, psT)
                    cur_atT[h] = atT

            def emit_outproj(q, lcs):
                at0, at1 = cur_atT[0], cur_atT[1]
                for lc in lcs:
                    ops = psS.tile([128, 2, QB], F32, tag="st", name="ops")
                    osb = obp.tile([128, D], F32, tag="ob", name="osb")
                    for n in range(2):
                        ns = slice(QB * n, QB * (n + 1))
                        nc.tensor.matmul(ops[:, n, :], at0[:, lc, :],
                                         wo0_sb[:, ns], start=True, stop=False)
                        nc.tensor.matmul(ops[:, n, :], at1[:, lc, :],
                                         wo1_sb[:, ns], start=False, stop=True)
                    nc.vector.tensor_copy(osb, ops)
                    nc.sync.dma_start(
                        out=out[QB * q + KT * lc:QB * q + KT * (lc + 1), :],
                        in_=osb)

            # ---- software-pipelined emission ----
            # prologue: enough K/V for group 0, Q-proj 0, scores(0, 0)
            emit_kproj(0)                      # k-tiles 0-3
            for l in range(3):
                emit_vproj(l)
            kdone, vdone = 1, 3
            emit_qproj(0)
            emit_scores(0, 0)

            # holds (epi_q, atT deferral) state
            for q in range(NQB):
                for g in range(NG):
                    emit_exp(q, g)
                    # JIT K/V projections during block 0
                    if q == 0 and g + 1 < NG:
                        k1, gs1 = groups[g + 1]
                        need_k = min((k1 + gs1 + 3) // 4, NQB)
                        while kdone < need_k:
                            emit_kproj(kdone)
                            kdone += 1
                        while vdone < min(k1 + gs1, NKT):
                            emit_vproj(vdone)
                            vdone += 1
                    if g + 1 < NG:
                        emit_scores(q, g + 1)
                    elif q + 1 < NQB:
                        emit_qproj(q + 1)
                        emit_scores(q + 1, 0)
                    if g == 0:
                        if q > 0:
                            emit_epilogue()
                        emit_zero_av()
                    emit_pv(g)
                    if q > 0 and 1 <= g <= 4:
                        emit_outproj(q - 1, (g - 1,))
            emit_epilogue()
            emit_outproj(NQB - 1, (0, 1, 2, 3))
    nc.finalize()
    return nc


def _prep_inputs(x, Wq, bq, Wk, bk, Wv, bv, Wo, bo):
    bf = ml_dtypes.bfloat16
    xT = np.ascontiguousarray(np.asarray(x, dtype=np.float32)[0].T).astype(bf)
    Wq = np.asarray(Wq, dtype=np.float32)
    Wk = np.asarray(Wk, dtype=np.float32)
    Wv = np.asarray(Wv, dtype=np.float32)
    Wo = np.asarray(Wo, dtype=np.float32)
    bq = np.asarray(bq, dtype=np.float32)
    bk = np.asarray(bk, dtype=np.float32)
    bv = np.asarray(bv, dtype=np.float32)
    has_bias = bool(np.any(bq) or np.any(bk) or np.any(bv))
    ident = np.eye(128, dtype=np.float32).astype(bf)
    in_maps = []
    for c in range(NCORES):
        qsl = slice(HPC * HD * c, HPC * HD * (c + 1))   # this core's q-head cols
        kv = c // 2                                     # its kv head
        ksl = slice(HD * kv, HD * (kv + 1))
        wk_c = np.ascontiguousarray(Wk[:, ksl])
        in_maps.append({
            "xT": xT,
            "wq": np.ascontiguousarray(Wq[:, qsl]).astype(bf),
            "wk": np.concatenate([wk_c, wk_c], axis=1).astype(bf),
            "wv": np.ascontiguousarray(Wv[:, ksl]).astype(bf),
            "wo0": np.ascontiguousarray(
                Wo[HPC * HD * c:HPC * HD * c + HD, :]).astype(bf),
            "wo1": np.ascontiguousarray(
                Wo[HPC * HD * c + HD:HPC * HD * (c + 1), :]).astype(bf),
            "ident": ident,
            "bq": bq[qsl].reshape(1, -1).astype(bf),
            "bk": np.concatenate([bk[ksl], bk[ksl]]).reshape(1, -1).astype(bf),
            "bv": bv[ksl].reshape(1, -1).astype(bf),
        })
    return in_maps, has_bias


def run(inputs, trace=False):
    in_maps, has_bias = _prep_inputs(**inputs)
    key = ("nc", has_bias)
    if key not in _CACHE:
        _CACHE[key] = _build(has_bias)
    nc = _CACHE[key]
    res = run_bass_kernel_spmd(nc, in_maps, list(range(NCORES)), trace=trace)
    bo = np.asarray(inputs["bo"], dtype=np.float32)
    acc = np.zeros((L, D), dtype=np.float32)
    for r in res.results:
        acc += np.asarray(r["out"], dtype=np.float32)
    out = (acc + bo).reshape(1, L, D)
    return out, res


def kernel(**inputs):
    out, _ = run(inputs, trace=False)
    return out
